# revision 34
# baseline (speedup 1.0000x reference)
"""Trainium2 Bass kernel for nn_CubicalModel_ISM.

Reference computes Xp = X @ p and Yp = Y @ p (X, Y: [784, 32768] f32,
p: [32768] f32) and gathers 100 (i, j) positions from each reshaped
[28, 28] image.  Only the gathered rows matter: inds1/inds2 give <=100
unique rows of X and of Y (R = n1 + n2 ~ 187 of 1568 total), so the
device only needs R dot products of length 32768.

Sharding: q (parameter) axis on the PARTITION axis, 32 q-chunks of 128
per core.  The q axis is first permuted into p-sorted order (a pure
reindexing; dots are permutation invariant), then the 256 sorted
128-col chunks are dealt round-robin to the 8 cores (core = g % 8), so
every core gets an identical mix of chunk precisions (SPMD) and equal
bytes.

Precision (the whole trick): a raw bf16 cast of X fails the 2e-2 gate
outright (0.25 max rel err), but quantization error is controllable.
Each row is quantized with TWO GLOBAL error-feedback carry chains over
the sorted-p axis - negative side ascending, positive side descending,
both terminating at the p~0 crossing - so the dot error telescopes to
sum_i carry_i * (p_i - p_{i-1}) over the tiny sorted gaps and the
chain-end boundary term multiplies |p| ~ 0.  That absorbs even fp8
rounding: the 224 chunks whose |p-hat| is smallest ship as fp8-e4m3
(1B) and only the 32 tail chunks (largest |p|) ship as bf16 (2B),
0.56x the bytes of all-bf16.  Measured max rel err on HW: 1.8e-3
(all-bf16 feedback: 1.5e-3; fp8 without the global chains: 9e-2).
p itself ships exactly as bf16 hi + lo halves (rebuilds to ~2^-18).

Compute: PE matvec.  Per chunk the stationary operand is ph[:, 2m:2m+2]
= [p_hi | p_lo] bf16 (LDWEIGHTS cost scales with stationary COLUMNS =
2), the moving operand is the chunk's R2 columns - bf16 slice or a
bitcast-to-fp8 view of the byte-packed tile (fp8 moving streams at the
same 1 col/cycle; no DoubleRow needed since weights stay bf16).  All
32 matmuls accumulate into PSUM [2, R2] f32 (hi row, lo row; host adds
them).  DVE copies PSUM->SBUF, a 2-line DMA returns it, host does the
8-core reduce and the tiny unique-inverse gather.

Schedule (measured): ~0.61us HWDGE descriptor-gen per dma_start
serialized on Sync, ~280 GB/s/core stream with all 8 cores (HBM
contention bound), ~0.9us completion-sem receipt, PE 155ns/chunk cold
vs 80ns warm (HAM un-throttles 1.2->2.4 GHz after ~3.4us of sustained
PE busy).  Pieces decrease to a 1-chunk tail; warm-up matmuls bridge
the PE from loop entry to piece 0's sem so the HAM window opens during
the real chain.  ~8us of fixed NEFF-wrapper teardown (all 5 engines
zero the whole semaphore file individually) plus ~1.2us entry and the
~2.2us copy+out-DMA latency chain are outside kernel control.
"""

import numpy as np

H = W = 28
Q = 32768
N_CORES = 8
QS = Q // N_CORES   # 4096 q per core
NJ = QS // 128      # 32 q-chunks of 128 per core (PE contraction dim)
NCHUNK = 256        # global sorted 128-col chunks
F8 = 30             # fp8 chunks per core; NJ - F8 = 2 bf16 chunks
NB = NJ - F8
# Stream pieces as (bf16 chunks, fp8 chunks); piece 0 also carries ph.
# Decreasing byte sizes to a 1-chunk tail (kernel tail = last piece's
# completion sem + its matmuls); 6 issues keeps the 0.61us/issue HWDGE
# descriptor-gen from gating the now-shorter stream.
PIECES = [(NB, 5), (0, 9), (0, 8), (0, 5), (0, 3)]
NP = len(PIECES)
PHB = 4 * NJ        # ph bytes per partition at the head of piece 0
# PE column tiling: chunk j runs in array column group j % COLG
# (tile_position=(0, 32*(j%COLG))), each group fed by its own XBUS, so
# COLG matmuls execute concurrently (measured ~4ns stagger).  The
# 32-chunk chain collapses to NJ/COLG wave times, which keeps it
# completion-sem-bound even at the cold 1.2 GHz PE clock - no HAM
# warm-up needed at all.  Measured wave pitch is one cold MM (~157ns):
# COLG=2's 16 waves (2.5us) put the chain back on the critical path,
# while COLG=3's 11 waves (1.7us) hide under the stream; COLG=3 also
# keeps the result footprint at [66, R2] (vs COLG=4's [98, R2] whose
# out-DMA descriptor-gen cost ~0.3us more) and stays off array
# quadrant 3.
COLG = 3

_CACHE = {}


def _build_nc(R2):
    import concourse.bacc as bacc
    import concourse.mybir as mybir
    from concourse.tile import TileContext

    nc = bacc.Bacc(None, enable_partition_id=False)
    f32 = mybir.dt.float32
    bf16 = mybir.dt.bfloat16
    fp8 = mybir.dt.float8e4

    def piece_bytes(k):
        b, f = PIECES[k]
        return (PHB if k == 0 else 0) + b * 2 * R2 + f * R2

    # Tensors are declared bf16 but hold byte-packed mixed content; fp8
    # chunk slices are bitcast back to fp8 for the matmul rhs.  R2 is
    # even so every chunk block lands on a 2-byte boundary.
    sels = [
        nc.dram_tensor(f"sel{k}", [128, piece_bytes(k) // 2], bf16,
                       kind="ExternalInput")
        for k in range(NP)
    ]
    # col-group g's [hi; lo] accumulator lives at psum partitions
    # 32g, 32g+1; the copy/out ship partitions 0..97 wholesale (one DVE
    # copy + one DMA beat a partition-gather).
    OP = 32 * (COLG - 1) + 2
    out = nc.dram_tensor("out", [OP, R2], f32, kind="ExternalOutput")

    with TileContext(nc) as tc:
        with (
            tc.tile_pool(name="pieces", bufs=1) as piece_pool,
            tc.tile_pool(name="respool", bufs=1) as res_pool,
            tc.tile_pool(name="psum", bufs=1, space="PSUM") as psum_pool,
        ):
            pieces = [
                piece_pool.tile(
                    [128, piece_bytes(k) // 2], bf16,
                    tag=f"piece{k}", name=f"piece{k}",
                )
                for k in range(NP)
            ]
            # Alternate the piece issues between the two HWDGE queues
            # (Sync and Scalar): descriptor-gen is ~0.65us per dma_start
            # and serialized per engine, so 5 issues on one queue
            # (3.25us) would pace the ~3.2us stream tail.  The SDMA
            # engines round-robin both queues at packet granularity, so
            # total stream time is unchanged and per-piece sems keep the
            # matmul waits correct regardless of completion order.
            for k in range(NP):
                eng = nc.sync if k % 2 == 0 else nc.scalar
                eng.dma_start(out=pieces[k][:, :], in_=sels[k][:, :])
            acc = psum_pool.tile([128, R2], f32)
            m = 0
            for k in range(NP):
                nb, nf = PIECES[k]
                off = (PHB // 2) if k == 0 else 0  # bf16-col offset
                for is8 in [False] * nb + [True] * nf:
                    if is8:
                        rhs = pieces[k][:, off : off + R2 // 2].bitcast(fp8)
                        off += R2 // 2
                    else:
                        rhs = pieces[k][:, off : off + R2]
                        off += R2
                    g = m % COLG
                    nc.tensor.matmul(
                        acc[32 * g : 32 * g + 2, :],
                        pieces[0][:, 2 * m : 2 * m + 2],
                        rhs,
                        start=(m < COLG),
                        stop=(m >= NJ - COLG),
                        tile_position=(0, 32 * g),
                    )
                    m += 1
            res = res_pool.tile([OP, R2], f32)
            nc.vector.tensor_copy(res[:, :], acc[:OP, :])
            nc.sync.dma_start(out=out[:, :], in_=res[:, :])
    nc.finalize()
    return nc


def _get_nc(R2):
    if R2 not in _CACHE:
        _CACHE[R2] = _build_nc(R2)
    return _CACHE[R2]


def _unique_rows(inds):
    ij = np.asarray(inds).reshape(-1, 2).astype(np.int64)
    flat = ij[:, 0] * W + ij[:, 1]
    return np.unique(flat, return_inverse=True)


def _fp8_window(ps):
    """Contiguous 8-aligned window of 8*F8 sorted chunks (the fp8 region)
    minimizing the larger edge |p-hat|."""
    cmax = np.abs(ps.reshape(NCHUNK, 128)).max(axis=1)
    nf = 8 * F8
    best, besta = None, None
    for a in range(0, NCHUNK - nf + 1, 8):
        mx = max(cmax[a], cmax[a + nf - 1])
        if best is None or mx < best:
            best, besta = mx, a
    return besta, besta + nf


def _feedback_quant(M, ps, col_fp8, bf16, fp8):
    """Quantize M [R, Q] (columns in sorted-p order) elementwise to the
    dtype in col_fp8 (True -> fp8) with two global error-feedback carry
    chains per row, each running toward the p~0 crossing."""
    R = M.shape[0]
    z = int(np.searchsorted(ps, 0.0))
    out = np.empty((R, Q), dtype=np.float32)
    for lo, hi, step in [(0, z, 1), (Q - 1, z - 1, -1)]:
        carry = np.zeros(R, dtype=np.float32)
        for j in range(lo, hi, step):
            t = M[:, j] + carry
            q = (t.astype(fp8) if col_fp8[j] else t.astype(bf16)).astype(
                np.float32
            )
            carry = t - q
            out[:, j] = q
    return out


def _prep(X, Y, p, inds1, inds2):
    """Host prep: unique-row selection, p-sort, global feedback
    quantization, per-core byte-packed piece buffers."""
    import ml_dtypes

    bf16 = ml_dtypes.bfloat16
    fp8 = ml_dtypes.float8_e4m3
    X = np.asarray(X, dtype=np.float32)
    Y = np.asarray(Y, dtype=np.float32)
    p = np.asarray(p, dtype=np.float32)

    u1, inv1 = _unique_rows(inds1)
    u2, inv2 = _unique_rows(inds2)
    n1, n2 = len(u1), len(u2)
    R = n1 + n2
    R2 = R + (R & 1)

    p_hi16 = p.astype(bf16)
    p_lo16 = (p - p_hi16.astype(np.float32)).astype(bf16)
    p_rec = p_hi16.astype(np.float32) + p_lo16.astype(np.float32)

    order = np.argsort(p_rec, kind="stable")
    ps = p_rec[order]
    hi_s = p_hi16[order]
    lo_s = p_lo16[order]

    a8, b8 = _fp8_window(ps)
    chunk_fp8 = np.zeros(NCHUNK, dtype=bool)
    chunk_fp8[a8:b8] = True
    col_fp8 = np.repeat(chunk_fp8, 128)

    M = np.concatenate([X[u1], Y[u2]], axis=0)[:, order]  # [R, Q]
    Qv = _feedback_quant(M, ps, col_fp8, bf16, fp8)

    # Per core: global chunk g -> core g % 8, local position k = g // 8.
    # Stream order per core: bf16 chunks (k order) then fp8 chunks (k
    # order) - identical structure on every core because the window is
    # a contiguous run of 8*F8 chunks (8-aligned => same k range).
    kf0, kf1 = a8 // 8, a8 // 8 + F8
    stream_k = [k for k in range(NJ) if not (kf0 <= k < kf1)] + list(
        range(kf0, kf1)
    )
    in_maps = []
    for c in range(N_CORES):
        bufs = []
        for k in range(NP):
            bufs.append(
                np.zeros(
                    (128, (PHB if k == 0 else 0)
                     + PIECES[k][0] * 2 * R2 + PIECES[k][1] * R2),
                    dtype=np.uint8,
                )
            )
        # ph head of piece 0, columns in stream order
        ph = np.empty((128, 2 * NJ), dtype=bf16)
        for m, k in enumerate(stream_k):
            g = c + 8 * k
            ph[:, 2 * m] = hi_s[g * 128 : (g + 1) * 128]
            ph[:, 2 * m + 1] = lo_s[g * 128 : (g + 1) * 128]
        bufs[0][:, :PHB] = ph.view(np.uint8)
        # chunk blocks, byte-packed: [128, R2] per chunk, transposed so
        # q is the partition axis
        m = 0
        for k in range(NP):
            off = PHB if k == 0 else 0
            nb, nf = PIECES[k]
            for is8 in [False] * nb + [True] * nf:
                g = c + 8 * stream_k[m]
                blk = np.zeros((128, R2), dtype=fp8 if is8 else bf16)
                blk[:, :R] = Qv[:, g * 128 : (g + 1) * 128].T
                raw = blk.view(np.uint8)
                bufs[k][:, off : off + raw.shape[1]] = raw
                off += raw.shape[1]
                m += 1
        in_maps.append(
            {f"sel{k}": bufs[k].view(bf16) for k in range(NP)}
        )

    nc = _get_nc(R2)
    return nc, in_maps, (n1, n2, inv1, inv2, R, R2)


def kernel(X, Y, p, inds1, inds2):
    from concourse.bass_utils import run_bass_kernel_spmd

    nc, in_maps, (n1, n2, inv1, inv2, R, R2) = _prep(X, Y, p, inds1, inds2)
    results = run_bass_kernel_spmd(
        nc, in_maps, list(range(N_CORES))
    ).results

    total = np.zeros(R2, dtype=np.float32)
    for c in range(N_CORES):
        o = results[c]["out"]  # [98, R2]: col-group g's hi/lo at 32g, 32g+1
        for g in range(COLG):
            total += o[32 * g] + o[32 * g + 1]

    dgm1 = total[:n1][inv1].reshape(-1, 2).astype(np.float32, copy=False)
    dgm2 = total[n1 : n1 + n2][inv2].reshape(-1, 2).astype(
        np.float32, copy=False
    )
    return dgm1, dgm2


# revision 36
# speedup vs baseline: 1.1363x; 1.1363x over previous
"""Trainium2 Bass kernel for nn_CubicalModel_ISM.

Reference computes Xp = X @ p and Yp = Y @ p (X, Y: [784, 32768] f32,
p: [32768] f32) and gathers 100 (i, j) positions from each reshaped
[28, 28] image.  Only the gathered rows matter: inds1/inds2 give <=100
unique rows of X and of Y (R = n1 + n2 ~ 187 of 1568 total), so the
device only needs R dot products of length 32768.

Sharding: q (parameter) axis on the PARTITION axis, 32 q-chunks of 128
per core.  The q axis is first permuted into p-sorted order (a pure
reindexing; dots are permutation invariant), then the 256 sorted
128-col chunks are dealt round-robin to the 8 cores (core = g % 8), so
every core gets an identical mix of chunk precisions (SPMD) and equal
bytes.

Precision (the whole trick): a raw bf16 cast of X fails the 2e-2 gate
outright (0.25 max rel err), but quantization error is controllable.
Each row is quantized with TWO GLOBAL error-feedback carry chains over
the sorted-p axis - negative side ascending, positive side descending,
both terminating at the p~0 crossing - so the dot error telescopes to
sum_i carry_i * (p_i - p_{i-1}) over the tiny sorted gaps and the
chain-end boundary term multiplies |p| ~ 0.  That absorbs even fp8
rounding: the 224 chunks whose |p-hat| is smallest ship as fp8-e4m3
(1B) and only the 16 tail chunks (largest |p|) ship as bf16 (2B),
0.53x the bytes of all-bf16.  Measured max rel err on HW: 1.9e-3
(all-bf16 feedback: 1.5e-3; fp8 without the global chains: 9e-2).
p itself ships exactly as bf16 hi + lo halves (rebuilds to ~2^-18).

Compute: PE matvec with column tiling.  Per chunk the stationary
operand is ph[:, 2m:2m+2] = [p_hi | p_lo] bf16 (LDWEIGHTS cost scales
with stationary COLUMNS = 2), the moving operand is the chunk's R2
columns - bf16 slice or a bitcast-to-fp8 view of the byte-packed tile
(fp8 moving streams at the same 1 col/cycle; no DoubleRow needed since
weights stay bf16).  Chunk m runs in array column group m % 3 via
tile_position=(0, 32g), each group fed by its own XBUS, so 3 matmuls
execute concurrently (~4ns stagger): the 32-chunk chain is ~11 wave
times and stays completion-sem-bound even at the cold 1.2 GHz PE
clock, so no HAM warm-up is needed at all.  Group g accumulates into
PSUM [32g:32g+2, :] f32; DVE copies [66, R2] PSUM->SBUF, one DMA
returns it, host adds the 6 hi/lo rows per core, reduces over cores,
and applies the tiny unique-inverse gather.

Schedule (measured): ~0.65us HWDGE descriptor-gen per dma_start,
serialized per engine - so the piece issues alternate between the Sync
and Scalar HWDGE queues to keep descriptor-gen off the ~3us stream's
critical path (SDMA round-robins both queues at packet granularity;
total stream time unchanged; per-piece sems make completion order
irrelevant).  ~280 GB/s/core stream with all 8 cores streaming (HBM
contention bound), ~0.9us completion-sem receipt.  Piece sizes
decrease to a small tail so almost nothing is un-hidden after the
final sem.  ~8us of fixed NEFF-wrapper teardown (all 5 engines zero
the whole semaphore file individually) plus ~1.2us entry and the
~2.3us copy+out-DMA latency chain are outside kernel control.  Whole-
chip ~15-20%% downclocks (P0 power state) appear under sustained
back-to-back benching; DMA-issue durations are the clock proxy.
"""

import numpy as np

H = W = 28
Q = 32768
N_CORES = 8
QS = Q // N_CORES   # 4096 q per core
NJ = QS // 128      # 32 q-chunks of 128 per core (PE contraction dim)
NCHUNK = 256        # global sorted 128-col chunks
F8 = 30             # fp8 chunks per core; NJ - F8 = 2 bf16 chunks
NB = NJ - F8
# Stream pieces as (bf16 chunks, fp8 chunks); piece 0 also carries ph.
# Decreasing byte sizes to a 1-chunk tail (kernel tail = last piece's
# completion sem + its matmuls); 6 issues keeps the 0.61us/issue HWDGE
# descriptor-gen from gating the now-shorter stream.
PIECES = [(NB, 5), (0, 9), (0, 8), (0, 5), (0, 3)]
NP = len(PIECES)
PHB = 4 * NJ        # ph bytes per partition at the head of piece 0
# PE column tiling: chunk j runs in array column group j % COLG
# (tile_position=(0, 32*(j%COLG))), each group fed by its own XBUS, so
# COLG matmuls execute concurrently (measured ~4ns stagger).  The
# 32-chunk chain collapses to NJ/COLG wave times, which keeps it
# completion-sem-bound even at the cold 1.2 GHz PE clock - no HAM
# warm-up needed at all.  Measured wave pitch is one cold MM (~157ns):
# COLG=2's 16 waves (2.5us) put the chain back on the critical path,
# while COLG=3's 11 waves (1.7us) hide under the stream; COLG=3 also
# keeps the result footprint at [66, R2] (vs COLG=4's [98, R2] whose
# out-DMA descriptor-gen cost ~0.3us more) and stays off array
# quadrant 3.
COLG = 3

_CACHE = {}


def _build_nc(R2):
    import concourse.bacc as bacc
    import concourse.mybir as mybir
    from concourse.tile import TileContext

    nc = bacc.Bacc(None, enable_partition_id=False)
    f32 = mybir.dt.float32
    bf16 = mybir.dt.bfloat16
    fp8 = mybir.dt.float8e4

    def piece_bytes(k):
        b, f = PIECES[k]
        return (PHB if k == 0 else 0) + b * 2 * R2 + f * R2

    # Tensors are declared bf16 but hold byte-packed mixed content; fp8
    # chunk slices are bitcast back to fp8 for the matmul rhs.  R2 is
    # even so every chunk block lands on a 2-byte boundary.
    sels = [
        nc.dram_tensor(f"sel{k}", [128, piece_bytes(k) // 2], bf16,
                       kind="ExternalInput")
        for k in range(NP)
    ]
    # col-group g's [hi; lo] accumulator lives at psum partitions
    # 32g, 32g+1; the copy/out ship partitions 0..97 wholesale (one DVE
    # copy + one DMA beat a partition-gather).
    OP = 32 * (COLG - 1) + 2
    out = nc.dram_tensor("out", [OP, R2], f32, kind="ExternalOutput")

    with TileContext(nc) as tc:
        with (
            tc.tile_pool(name="pieces", bufs=1) as piece_pool,
            tc.tile_pool(name="respool", bufs=1) as res_pool,
            tc.tile_pool(name="psum", bufs=1, space="PSUM") as psum_pool,
        ):
            pieces = [
                piece_pool.tile(
                    [128, piece_bytes(k) // 2], bf16,
                    tag=f"piece{k}", name=f"piece{k}",
                )
                for k in range(NP)
            ]
            # Alternate the piece issues between the two HWDGE queues
            # (Sync and Scalar): descriptor-gen is ~0.65us per dma_start
            # and serialized per engine, so 5 issues on one queue
            # (3.25us) would pace the ~3.2us stream tail.  The SDMA
            # engines round-robin both queues at packet granularity, so
            # total stream time is unchanged and per-piece sems keep the
            # matmul waits correct regardless of completion order.
            for k in range(NP):
                eng = nc.sync if k % 2 == 0 else nc.scalar
                eng.dma_start(out=pieces[k][:, :], in_=sels[k][:, :])
            acc = psum_pool.tile([128, R2], f32)
            m = 0
            for k in range(NP):
                nb, nf = PIECES[k]
                off = (PHB // 2) if k == 0 else 0  # bf16-col offset
                for is8 in [False] * nb + [True] * nf:
                    if is8:
                        rhs = pieces[k][:, off : off + R2 // 2].bitcast(fp8)
                        off += R2 // 2
                    else:
                        rhs = pieces[k][:, off : off + R2]
                        off += R2
                    g = m % COLG
                    nc.tensor.matmul(
                        acc[32 * g : 32 * g + 2, :],
                        pieces[0][:, 2 * m : 2 * m + 2],
                        rhs,
                        start=(m < COLG),
                        stop=(m >= NJ - COLG),
                        tile_position=(0, 32 * g),
                    )
                    m += 1
            res = res_pool.tile([OP, R2], f32)
            nc.vector.tensor_copy(res[:, :], acc[:OP, :])
            nc.sync.dma_start(out=out[:, :], in_=res[:, :])
    nc.finalize()
    return nc


def _get_nc(R2):
    if R2 not in _CACHE:
        _CACHE[R2] = _build_nc(R2)
    return _CACHE[R2]


def _unique_rows(inds):
    ij = np.asarray(inds).reshape(-1, 2).astype(np.int64)
    flat = ij[:, 0] * W + ij[:, 1]
    return np.unique(flat, return_inverse=True)


def _fp8_window(ps):
    """Contiguous 8-aligned window of 8*F8 sorted chunks (the fp8 region)
    minimizing the larger edge |p-hat|."""
    cmax = np.abs(ps.reshape(NCHUNK, 128)).max(axis=1)
    nf = 8 * F8
    best, besta = None, None
    for a in range(0, NCHUNK - nf + 1, 8):
        mx = max(cmax[a], cmax[a + nf - 1])
        if best is None or mx < best:
            best, besta = mx, a
    return besta, besta + nf


def _feedback_quant(M, ps, col_fp8, bf16, fp8):
    """Quantize M [R, Q] (columns in sorted-p order) elementwise to the
    dtype in col_fp8 (True -> fp8) with two global error-feedback carry
    chains per row, each running toward the p~0 crossing."""
    R = M.shape[0]
    z = int(np.searchsorted(ps, 0.0))
    out = np.empty((R, Q), dtype=np.float32)
    for lo, hi, step in [(0, z, 1), (Q - 1, z - 1, -1)]:
        carry = np.zeros(R, dtype=np.float32)
        for j in range(lo, hi, step):
            t = M[:, j] + carry
            q = (t.astype(fp8) if col_fp8[j] else t.astype(bf16)).astype(
                np.float32
            )
            carry = t - q
            out[:, j] = q
    return out


def _prep(X, Y, p, inds1, inds2):
    """Host prep: unique-row selection, p-sort, global feedback
    quantization, per-core byte-packed piece buffers."""
    import ml_dtypes

    bf16 = ml_dtypes.bfloat16
    fp8 = ml_dtypes.float8_e4m3
    X = np.asarray(X, dtype=np.float32)
    Y = np.asarray(Y, dtype=np.float32)
    p = np.asarray(p, dtype=np.float32)

    u1, inv1 = _unique_rows(inds1)
    u2, inv2 = _unique_rows(inds2)
    n1, n2 = len(u1), len(u2)
    R = n1 + n2
    R2 = R + (R & 1)

    p_hi16 = p.astype(bf16)
    p_lo16 = (p - p_hi16.astype(np.float32)).astype(bf16)
    p_rec = p_hi16.astype(np.float32) + p_lo16.astype(np.float32)

    order = np.argsort(p_rec, kind="stable")
    ps = p_rec[order]
    hi_s = p_hi16[order]
    lo_s = p_lo16[order]

    a8, b8 = _fp8_window(ps)
    chunk_fp8 = np.zeros(NCHUNK, dtype=bool)
    chunk_fp8[a8:b8] = True
    col_fp8 = np.repeat(chunk_fp8, 128)

    M = np.concatenate([X[u1], Y[u2]], axis=0)[:, order]  # [R, Q]
    Qv = _feedback_quant(M, ps, col_fp8, bf16, fp8)

    # Per core: global chunk g -> core g % 8, local position k = g // 8.
    # Stream order per core: bf16 chunks (k order) then fp8 chunks (k
    # order) - identical structure on every core because the window is
    # a contiguous run of 8*F8 chunks (8-aligned => same k range).
    kf0, kf1 = a8 // 8, a8 // 8 + F8
    stream_k = [k for k in range(NJ) if not (kf0 <= k < kf1)] + list(
        range(kf0, kf1)
    )
    in_maps = []
    for c in range(N_CORES):
        bufs = []
        for k in range(NP):
            bufs.append(
                np.zeros(
                    (128, (PHB if k == 0 else 0)
                     + PIECES[k][0] * 2 * R2 + PIECES[k][1] * R2),
                    dtype=np.uint8,
                )
            )
        # ph head of piece 0, columns in stream order
        ph = np.empty((128, 2 * NJ), dtype=bf16)
        for m, k in enumerate(stream_k):
            g = c + 8 * k
            ph[:, 2 * m] = hi_s[g * 128 : (g + 1) * 128]
            ph[:, 2 * m + 1] = lo_s[g * 128 : (g + 1) * 128]
        bufs[0][:, :PHB] = ph.view(np.uint8)
        # chunk blocks, byte-packed: [128, R2] per chunk, transposed so
        # q is the partition axis
        m = 0
        for k in range(NP):
            off = PHB if k == 0 else 0
            nb, nf = PIECES[k]
            for is8 in [False] * nb + [True] * nf:
                g = c + 8 * stream_k[m]
                blk = np.zeros((128, R2), dtype=fp8 if is8 else bf16)
                blk[:, :R] = Qv[:, g * 128 : (g + 1) * 128].T
                raw = blk.view(np.uint8)
                bufs[k][:, off : off + raw.shape[1]] = raw
                off += raw.shape[1]
                m += 1
        in_maps.append(
            {f"sel{k}": bufs[k].view(bf16) for k in range(NP)}
        )

    nc = _get_nc(R2)
    return nc, in_maps, (n1, n2, inv1, inv2, R, R2)


def kernel(X, Y, p, inds1, inds2):
    from concourse.bass_utils import run_bass_kernel_spmd

    nc, in_maps, (n1, n2, inv1, inv2, R, R2) = _prep(X, Y, p, inds1, inds2)
    results = run_bass_kernel_spmd(
        nc, in_maps, list(range(N_CORES))
    ).results

    total = np.zeros(R2, dtype=np.float32)
    for c in range(N_CORES):
        o = results[c]["out"]  # [98, R2]: col-group g's hi/lo at 32g, 32g+1
        for g in range(COLG):
            total += o[32 * g] + o[32 * g + 1]

    dgm1 = total[:n1][inv1].reshape(-1, 2).astype(np.float32, copy=False)
    dgm2 = total[n1 : n1 + n2][inv2].reshape(-1, 2).astype(
        np.float32, copy=False
    )
    return dgm1, dgm2
